# revision 13
# baseline (speedup 1.0000x reference)
"""Per-token sparse MoE kernel for Trainium2 (8 NeuronCores, Bass/Tile).

Problem: y[b,t,:] = sum_e relu(x[b,t]@gw[t])[e] * (gelu(x[b,t]@W1[t,e]+b1)@W2[t,e]+b2)
Shapes: x[2048,16,128], W1[16,4,128,512], W2[16,4,512,128], gates[16,128,4].

Sharding: the t dimension (16) is split across the 8 cores (2 t-values per
core). That makes the problem embarrassingly parallel (no collectives) and
each core only loads its own 1/8 of the weights (~4.2 MB) instead of the
full 33 MB, so the kernel is PE-bound rather than HBM-bound.

Per-core dataflow, per t:
  x_t [B,D] --PE transpose--> xT [D,B]
  gate_T [E,B] = relu(gw^T @ xT)  (PE, gw stationary; ACT relu w/ bias)
  gate    [B,E]  by PE-transposing gate_T back (per 128-row block)
  h_T [H,B] = W1-slice^T @ xT     (PE, W1 stationary, 16 matmuls N=512)
  h = gelu(h_T + b1)              (ACT, exact-erf Gelu, per-partition bias)
  expert psum [Bblk,D] = h-block^T @ W2-block (PE, 4 accumulating matmuls)
  y += gate[:,e] * psum           (DVE tensor_scalar + adds)
  y += gate_T-block^T @ b2        (PE K=4 matmul; exact b2 handling)
"""

import contextlib
import ctypes
import sys
import types

import numpy as np

B, T, D, E, H = 2048, 16, 128, 4, 512
N_CORES = 8
T_LOC = T // N_CORES  # 2 t-values per core
NBLK = B // 128       # 16 b-blocks of 128
NCH = B // 512        # 4 b-chunks of 512 (matmul moving-operand max)

_CACHE: dict = {}


def _install_ntff_hook():
    """Provide antenv.axon_hooks (absent in this image) so that
    run_bass_kernel_spmd(trace=True) can capture NTFF profiles."""
    if "antenv.axon_hooks" in sys.modules:
        return
    try:
        lib = ctypes.CDLL("/opt/axon/libaxon_pjrt.so")
        if not hasattr(lib, "axon_start_nrt_profile"):
            hook = None
        else:
            lib.axon_start_nrt_profile.argtypes = [
                ctypes.POINTER(ctypes.c_int64),
                ctypes.c_size_t,
            ]
            lib.axon_start_nrt_profile.restype = ctypes.c_int64
            lib.axon_stop_nrt_profile.argtypes = [ctypes.c_char_p]
            lib.axon_stop_nrt_profile.restype = ctypes.c_int64

            @contextlib.contextmanager
            def hook(output_dir, device_ids):
                import jax

                jax.devices()
                if device_ids:
                    ids = (ctypes.c_int64 * len(device_ids))(*device_ids)
                    rc = lib.axon_start_nrt_profile(ids, len(device_ids))
                else:
                    rc = lib.axon_start_nrt_profile(None, 0)
                if rc != 0:
                    raise RuntimeError(f"axon_start_nrt_profile rc={rc}")
                try:
                    yield
                finally:
                    lib.axon_stop_nrt_profile(str(output_dir).encode())

        m = types.ModuleType("antenv.axon_hooks")
        m.get_axon_ntff_profile_hook = lambda: hook
        m.set_axon_ntff_profile_hook = lambda h: None
        sys.modules["antenv.axon_hooks"] = m
        import antenv

        antenv.axon_hooks = m
    except OSError:
        pass


def _build(dt_mm_name: str = "float32r"):
    """Build and compile the per-core Bass program. Same program on all cores.

    dt_mm_name selects the matmul-operand storage dtype:
      float32  — exact, but every matmul is a 2-pass HI/LO pair (slow)
      float32r — single-pass fp22-truncated reads (~1e-4 rel err)
      bfloat16 — single-pass + fast weight load (~5e-3 rel err)
    PSUM accumulation is fp32 in all cases.
    """
    import concourse.bass as bass
    import concourse.tile as tile
    from concourse import bacc, mybir

    dt_mm = getattr(mybir.dt, dt_mm_name)
    f32 = mybir.dt.float32
    # dtype for DRAM-resident matmul inputs: f32r shares fp32 bits so we can
    # declare DRAM as f32r (no cast); bf16 needs a casting (gpsimd) DMA.
    dt_dram = dt_mm if dt_mm != mybir.dt.bfloat16 else f32
    cast_load = dt_mm == mybir.dt.bfloat16
    AF = mybir.ActivationFunctionType

    nc = bacc.Bacc("TRN2", target_bir_lowering=False, debug=False, num_devices=N_CORES)

    x_d = nc.dram_tensor("x", [B, T_LOC, D], dt_dram, kind="ExternalInput").ap()
    w1_d = nc.dram_tensor("w1", [T_LOC, E, D, H], dt_dram, kind="ExternalInput").ap()
    b1_d = nc.dram_tensor("b1", [T_LOC, E, H], f32, kind="ExternalInput").ap()
    w2_d = nc.dram_tensor("w2", [T_LOC, E, H, D], dt_dram, kind="ExternalInput").ap()
    gw_d = nc.dram_tensor("gw", [T_LOC, D, E], dt_dram, kind="ExternalInput").ap()
    gb_d = nc.dram_tensor("gb", [T_LOC, E], f32, kind="ExternalInput").ap()
    id_d = nc.dram_tensor("ident", [128, 128], dt_dram, kind="ExternalInput").ap()
    y_d = nc.dram_tensor("y", [B, T_LOC, D], f32, kind="ExternalOutput").ap()
    if cast_load:
        # t-major bf16 staging copy of x so the hw DMA-transpose reads
        # contiguous [B, D] planes
        xbf_d = nc.dram_tensor("x_stage_bf16", [T_LOC, B, D], dt_mm).ap()

    def load(out_ap, in_ap):
        if cast_load:
            nc.gpsimd.dma_start(out_ap, in_ap)
        else:
            nc.sync.dma_start(out_ap, in_ap)

    with tile.TileContext(nc) as tc, contextlib.ExitStack() as ctx:
        ep = ctx.enter_context
        # SBUF pools
        const_p = ep(tc.tile_pool(name="const", bufs=1))
        x_p = ep(tc.tile_pool(name="x", bufs=2))
        xT_p = ep(tc.tile_pool(name="xT", bufs=2))
        h_p = ep(tc.tile_pool(name="h", bufs=2))
        w1_p = ep(tc.tile_pool(name="w1", bufs=3))
        w2_p = ep(tc.tile_pool(name="w2", bufs=3))
        y_p = ep(tc.tile_pool(name="y", bufs=2))
        tmp_p = ep(tc.tile_pool(name="tmp", bufs=4))
        gt_p = ep(tc.tile_pool(name="gt", bufs=2))
        small_p = ep(tc.tile_pool(name="small", bufs=4))
        # PSUM pools: hps 2x2 + sp 2 + tp 2 = 8 banks (gate psum shares "sp")
        hps_p = ep(tc.tile_pool(name="hps", bufs=2, space="PSUM"))
        sps_p = ep(tc.tile_pool(name="sps", bufs=2, space="PSUM"))
        tps_p = ep(tc.tile_pool(name="tps", bufs=2, space="PSUM"))

        ident = const_p.tile([128, 128], dt_mm)
        if cast_load:
            idf = const_p.tile([128, 128], f32)
            nc.sync.dma_start(idf[:], id_d[:])
            nc.gpsimd.tensor_copy(ident[:], idf[:])
        else:
            load(ident[:], id_d[:])

        if cast_load:
            # x -> bf16 -> DRAM bounce, per 512-row chunk: fast HWDGE fp32
            # load, GPSIMD SBUF cast, bf16 store. (SWDGE cast-DMA measured
            # ~43-85 GB/s; this path keeps the cast off DVE/ACT/PE.)
            for tl in range(T_LOC):
                for c in range(NCH):
                    xc_sb = x_p.tile([128, 512], f32, tag="xc")
                    nc.sync.dma_start(
                        xc_sb[:].rearrange("p (blk d) -> p blk d", blk=4),
                        x_d[512 * c : 512 * (c + 1), tl, :].rearrange(
                            "(blk p) d -> p blk d", p=128
                        ),
                    )
                    xb_sb = x_p.tile([128, 512], dt_mm, tag="xb")
                    nc.gpsimd.tensor_copy(xb_sb[:], xc_sb[:])
                    nc.sync.dma_start(
                        xbf_d[tl, 512 * c : 512 * (c + 1)].rearrange(
                            "(blk p) d -> p blk d", p=128
                        ),
                        xb_sb[:].rearrange("p (blk d) -> p blk d", blk=4),
                    )

        for tl in range(T_LOC):
            # ---- produce xT [d, b] ----
            xT = xT_p.tile([128, B], dt_mm, tag="xT")
            if cast_load:
                for c in range(NCH):
                    nc.sync.dma_start_transpose(
                        xT[:, 512 * c : 512 * (c + 1)],
                        xbf_d[tl, 512 * c : 512 * (c + 1)],
                    )
            else:
                x_sb = x_p.tile([128, B], dt_mm, tag="x")
                load(
                    x_sb[:].rearrange("p (blk d) -> p blk d", blk=NBLK),
                    x_d[:, tl, :].rearrange("(blk p) d -> p blk d", p=128),
                )
                for blk in range(NBLK):
                    tp = tps_p.tile([128, 128], dt_mm, tag="tp")
                    nc.tensor.transpose(
                        tp[:], x_sb[:, 128 * blk : 128 * (blk + 1)], ident[:]
                    )
                    nc.vector.tensor_copy(xT[:, 128 * blk : 128 * (blk + 1)], tp[:])

            # ---- gate_T [E, B] = relu(gw^T @ xT + gb) ----
            gw_sb = small_p.tile([128, E], dt_mm, tag="gw")
            if cast_load:
                gwf = small_p.tile([128, E], f32, tag="gwf")
                nc.sync.dma_start(gwf[:], gw_d[tl])
                nc.gpsimd.tensor_copy(gw_sb[:], gwf[:])
            else:
                load(gw_sb[:], gw_d[tl])
            gb_sb = small_p.tile([E, 1], f32, tag="gb")
            nc.sync.dma_start(gb_sb[:], gb_d[tl])
            gate_T = gt_p.tile([E, B], dt_mm, tag="gateT")
            for c in range(NCH):
                gps = sps_p.tile([E, 512], f32, tag="sp")
                nc.tensor.matmul(
                    gps[:], gw_sb[:], xT[:, 512 * c : 512 * (c + 1)],
                    start=True, stop=True,
                )
                nc.scalar.activation(
                    gate_T[:, 512 * c : 512 * (c + 1)], gps[:], AF.Relu,
                    bias=gb_sb[:, 0:1],
                )
            # gate [B, E] per-block by transposing gate_T back
            gate_sb = gt_p.tile([128, E * NBLK], f32, tag="gate")
            for blk in range(NBLK):
                tp = tps_p.tile([128, 128], dt_mm, tag="tp")
                nc.tensor.transpose(
                    tp[:, 0:E], gate_T[:, 128 * blk : 128 * (blk + 1)], ident[0:E, 0:E]
                )
                nc.vector.tensor_copy(gate_sb[:, E * blk : E * (blk + 1)], tp[:, 0:E])

            y_sb = y_p.tile([128, B], f32, tag="y")

            for e in range(E):
                if cast_load:
                    w1f = w1_p.tile([128, H], f32, tag="w1f")
                    nc.sync.dma_start(w1f[:], w1_d[tl, e])
                    w1_sb = w1_p.tile([128, H], dt_mm, tag="w1")
                    nc.gpsimd.tensor_copy(w1_sb[:], w1f[:])
                    w2f = w2_p.tile([128, H], f32, tag="w2f")
                    nc.sync.dma_start(
                        w2f[:].rearrange("p (hk d) -> p hk d", hk=4),
                        w2_d[tl, e].rearrange("(hk p) d -> p hk d", p=128),
                    )
                    w2_sb = w2_p.tile([128, H], dt_mm, tag="w2")
                    nc.gpsimd.tensor_copy(w2_sb[:], w2f[:])
                else:
                    w1_sb = w1_p.tile([128, H], dt_mm, tag="w1")
                    load(w1_sb[:], w1_d[tl, e])
                    w2_sb = w2_p.tile([128, H], dt_mm, tag="w2")
                    load(
                        w2_sb[:].rearrange("p (hk d) -> p hk d", hk=4),
                        w2_d[tl, e].rearrange("(hk p) d -> p hk d", p=128),
                    )
                b1_sb = small_p.tile([128, 4], f32, tag="b1")
                nc.sync.dma_start(
                    b1_sb[:].rearrange("p hb -> p hb"),
                    b1_d[tl, e].rearrange("(hb p) -> p hb", p=128),
                )

                # ---- h_T = gelu(W1slice^T @ xT + b1), laid out [128, (hb b)] ----
                h_sb = h_p.tile([128, 4 * B], dt_mm, tag="h")
                for hb in range(4):
                    for cc in range(2):  # two 1024-wide psum drains per hb
                        hps = hps_p.tile([128, 1024], f32, tag="hps")
                        for half in range(2):
                            c = 2 * cc + half
                            nc.tensor.matmul(
                                hps[:, 512 * half : 512 * (half + 1)],
                                w1_sb[:, 128 * hb : 128 * (hb + 1)],
                                xT[:, 512 * c : 512 * (c + 1)],
                                start=True, stop=True,
                            )
                        nc.scalar.activation(
                            h_sb[:, B * hb + 1024 * cc : B * hb + 1024 * (cc + 1)],
                            hps[:], AF.Gelu, bias=b1_sb[:, hb : hb + 1],
                        )

                # ---- expert out per 128-block, gated accumulate into y ----
                for g in range(4):  # groups of 4 blocks -> batched adds
                    if e > 0:
                        tmp = tmp_p.tile([128, 512], f32, tag="tmp")
                    else:
                        tmp = None
                    for j in range(4):
                        blk = 4 * g + j
                        yps = sps_p.tile([128, 128], f32, tag="sp")
                        for hk in range(4):
                            nc.tensor.matmul(
                                yps[:],
                                h_sb[:, B * hk + 128 * blk : B * hk + 128 * (blk + 1)],
                                w2_sb[:, 128 * hk : 128 * (hk + 1)],
                                start=(hk == 0), stop=(hk == 3),
                            )
                        gcol = gate_sb[:, E * blk + e : E * blk + e + 1]
                        if e == 0:
                            nc.vector.tensor_scalar(
                                y_sb[:, 512 * g + 128 * j : 512 * g + 128 * (j + 1)],
                                yps[:], gcol, None, bass.mybir.AluOpType.mult,
                            )
                        else:
                            nc.vector.tensor_scalar(
                                tmp[:, 128 * j : 128 * (j + 1)],
                                yps[:], gcol, None, bass.mybir.AluOpType.mult,
                            )
                    if e > 0:
                        nc.vector.tensor_add(
                            y_sb[:, 512 * g : 512 * (g + 1)],
                            y_sb[:, 512 * g : 512 * (g + 1)],
                            tmp[:],
                        )

            nc.sync.dma_start(
                y_d[:, tl, :].rearrange("(blk p) d -> p blk d", p=128),
                y_sb[:].rearrange("p (blk d) -> p blk d", blk=NBLK),
            )

    nc.compile()
    return nc


def get_program(dt_mm_name: str = "float32r"):
    key = ("nc", dt_mm_name)
    if key not in _CACHE:
        _install_ntff_hook()
        _CACHE[key] = _build(dt_mm_name)
    return _CACHE[key]


def make_in_maps(x, W1, b1, W2, b2, gate_w_infer, gate_b_infer):
    c = np.ascontiguousarray
    ident = np.eye(128, dtype=np.float32)
    maps = []
    for i in range(N_CORES):
        s = slice(T_LOC * i, T_LOC * (i + 1))
        maps.append(
            {
                "x": c(np.asarray(x, np.float32)[:, s, :]),
                "w1": c(np.asarray(W1, np.float32)[s]),
                "b1": c(np.asarray(b1, np.float32)[s]),
                "w2": c(np.asarray(W2, np.float32)[s]),
                "gw": c(np.asarray(gate_w_infer, np.float32)[s]),
                "gb": c(np.asarray(gate_b_infer, np.float32)[s]),
                "ident": ident,
            }
        )
    return maps


def kernel(x, W1, b1, W2, b2, gate_w_infer, gate_b_infer):
    from concourse.bass_utils import run_bass_kernel_spmd

    nc = get_program()
    maps = make_in_maps(x, W1, b1, W2, b2, gate_w_infer, gate_b_infer)
    res = run_bass_kernel_spmd(nc, maps, list(range(N_CORES)))
    y = np.concatenate([res.results[i]["y"] for i in range(N_CORES)], axis=1)
    b2 = np.asarray(b2, np.float32)
    if np.any(b2):
        # b2 is all-zero for this problem's setup_inputs; handled host-side
        # for generality since the device kernel omits the b2 term.
        xf = np.asarray(x, np.float32)
        gate = np.einsum("btd,tde->bte", xf, np.asarray(gate_w_infer, np.float32))
        gate = np.maximum(gate + np.asarray(gate_b_infer, np.float32), 0.0)
        y = y + np.einsum("bte,ted->btd", gate, b2)
    return y, np.asarray(0.0, dtype=np.float32)


# revision 14
# speedup vs baseline: 1.1626x; 1.1626x over previous
"""Per-token sparse MoE kernel for Trainium2 (8 NeuronCores, Bass/Tile).

Problem: y[b,t,:] = sum_e relu(x[b,t]@gw[t])[e] * (gelu(x[b,t]@W1[t,e]+b1)@W2[t,e]+b2)
Shapes: x[2048,16,128], W1[16,4,128,512], W2[16,4,512,128], gates[16,128,4].

Sharding: the t dimension (16) is split across the 8 cores (2 t-values per
core). That makes the problem embarrassingly parallel (no collectives) and
each core only loads its own 1/8 of the weights (~2.1 MB in bf16) instead
of the full 33 MB, so the kernel is compute-bound rather than HBM-bound.

Host-side marshalling (inside kernel(), part of sharding): inputs are
sliced per-core, cast to the matmul dtype, and x is pre-transposed to
xT[t, d, b] so the device program needs no transpose/cast machinery for
its inputs.

Per-core device dataflow, per t:
  gate_T [E,B] = relu(gw^T @ xT)  (PE, gw stationary; ACT relu w/ bias)
  gate    [B,E]  by PE-transposing gate_T back (per 128-column block)
  h_T [H,B] = W1-slice^T @ xT     (PE, W1 stationary, 16 matmuls N=512)
  h = gelu(h_T + b1)              (ACT, exact-erf Gelu, per-partition bias)
  expert psum [Bblk,D] = h-block^T @ W2-block (PE, 4 accumulating matmuls)
  y += gate[:,e] * psum           (DVE tensor_scalar + batched adds)

b2 is all-zero in this problem; a host-side numpy correction covers the
general case.
"""

import contextlib
import ctypes
import sys
import types

import numpy as np

B, T, D, E, H = 2048, 16, 128, 4, 512
N_CORES = 8
T_LOC = T // N_CORES  # 2 t-values per core
NBLK = B // 128       # 16 b-blocks of 128
NCH = B // 512        # 4 b-chunks of 512 (matmul moving-operand max)

_CACHE: dict = {}


def _install_ntff_hook():
    """Provide antenv.axon_hooks (absent in this image) so that
    run_bass_kernel_spmd(trace=True) can capture NTFF profiles."""
    if "antenv.axon_hooks" in sys.modules:
        return
    try:
        lib = ctypes.CDLL("/opt/axon/libaxon_pjrt.so")
        if not hasattr(lib, "axon_start_nrt_profile"):
            hook = None
        else:
            lib.axon_start_nrt_profile.argtypes = [
                ctypes.POINTER(ctypes.c_int64),
                ctypes.c_size_t,
            ]
            lib.axon_start_nrt_profile.restype = ctypes.c_int64
            lib.axon_stop_nrt_profile.argtypes = [ctypes.c_char_p]
            lib.axon_stop_nrt_profile.restype = ctypes.c_int64

            @contextlib.contextmanager
            def hook(output_dir, device_ids):
                import jax

                jax.devices()
                if device_ids:
                    ids = (ctypes.c_int64 * len(device_ids))(*device_ids)
                    rc = lib.axon_start_nrt_profile(ids, len(device_ids))
                else:
                    rc = lib.axon_start_nrt_profile(None, 0)
                if rc != 0:
                    raise RuntimeError(f"axon_start_nrt_profile rc={rc}")
                try:
                    yield
                finally:
                    lib.axon_stop_nrt_profile(str(output_dir).encode())

        m = types.ModuleType("antenv.axon_hooks")
        m.get_axon_ntff_profile_hook = lambda: hook
        m.set_axon_ntff_profile_hook = lambda h: None
        sys.modules["antenv.axon_hooks"] = m
        import antenv

        antenv.axon_hooks = m
    except OSError:
        pass


def _build(dt_mm_name: str = "bfloat16"):
    """Build and compile the per-core Bass program. Same program on all cores.

    dt_mm_name selects the matmul-operand dtype (host pre-casts inputs):
      float32  — exact, but every matmul is a 2-pass HI/LO pair (slow)
      float32r — single-pass fp22-truncated reads (~2.6e-4 rel err)
      bfloat16 — single-pass + fast weight load (~4e-3 rel err)
    PSUM accumulation is fp32 in all cases.
    """
    import concourse.bass as bass
    import concourse.tile as tile
    from concourse import bacc, mybir

    dt_mm = getattr(mybir.dt, dt_mm_name)
    f32 = mybir.dt.float32
    AF = mybir.ActivationFunctionType

    nc = bacc.Bacc("TRN2", target_bir_lowering=False, debug=False, num_devices=N_CORES)

    xT_d = nc.dram_tensor("xT", [T_LOC, D, B], dt_mm, kind="ExternalInput").ap()
    w1_d = nc.dram_tensor("w1", [T_LOC, E, D, H], dt_mm, kind="ExternalInput").ap()
    b1_d = nc.dram_tensor("b1t", [T_LOC, E, 128, 4], f32, kind="ExternalInput").ap()
    w2_d = nc.dram_tensor("w2", [T_LOC, E, H, D], dt_mm, kind="ExternalInput").ap()
    gw_d = nc.dram_tensor("gw", [T_LOC, D, E], dt_mm, kind="ExternalInput").ap()
    gb_d = nc.dram_tensor("gb", [T_LOC, E], f32, kind="ExternalInput").ap()
    id_d = nc.dram_tensor("ident", [E, E], dt_mm, kind="ExternalInput").ap()
    y_d = nc.dram_tensor("y", [B, T_LOC, D], f32, kind="ExternalOutput").ap()

    with tile.TileContext(nc) as tc, contextlib.ExitStack() as ctx:
        ep = ctx.enter_context
        # SBUF pools
        const_p = ep(tc.tile_pool(name="const", bufs=1))
        xT_p = ep(tc.tile_pool(name="xT", bufs=2))
        h_p = ep(tc.tile_pool(name="h", bufs=2))
        w1_p = ep(tc.tile_pool(name="w1", bufs=3))
        w2_p = ep(tc.tile_pool(name="w2", bufs=3))
        y_p = ep(tc.tile_pool(name="y", bufs=2))
        tmp_p = ep(tc.tile_pool(name="tmp", bufs=4))
        gt_p = ep(tc.tile_pool(name="gt", bufs=2))
        small_p = ep(tc.tile_pool(name="small", bufs=4))
        # PSUM pools: hps 2x2 + sp 2 + tp 2 = 8 banks (gate psum shares "sp")
        hps_p = ep(tc.tile_pool(name="hps", bufs=2, space="PSUM"))
        sps_p = ep(tc.tile_pool(name="sps", bufs=2, space="PSUM"))
        tps_p = ep(tc.tile_pool(name="tps", bufs=2, space="PSUM"))

        ident = const_p.tile([E, E], dt_mm)
        nc.sync.dma_start(ident[:], id_d[:])

        for tl in range(T_LOC):
            # ---- xT [d, b]: chunked contiguous loads (pre-transposed on host)
            xT = xT_p.tile([128, B], dt_mm, tag="xT")
            for c in range(NCH):
                nc.sync.dma_start(
                    xT[:, 512 * c : 512 * (c + 1)], xT_d[tl, :, 512 * c : 512 * (c + 1)]
                )

            # ---- gate_T [E, B] = relu(gw^T @ xT + gb) ----
            gw_sb = small_p.tile([128, E], dt_mm, tag="gw")
            nc.sync.dma_start(gw_sb[:], gw_d[tl])
            gb_sb = small_p.tile([E, 1], f32, tag="gb")
            nc.sync.dma_start(gb_sb[:], gb_d[tl])
            gate_T = gt_p.tile([E, B], dt_mm, tag="gateT")
            for c in range(NCH):
                gps = sps_p.tile([E, 512], f32, tag="sp")
                nc.tensor.matmul(
                    gps[:], gw_sb[:], xT[:, 512 * c : 512 * (c + 1)],
                    start=True, stop=True,
                )
                nc.scalar.activation(
                    gate_T[:, 512 * c : 512 * (c + 1)], gps[:], AF.Relu,
                    bias=gb_sb[:, 0:1],
                )
            # gate [B, E] per-block by PE-transposing gate_T back
            gate_sb = gt_p.tile([128, E * NBLK], f32, tag="gate")
            for blk in range(NBLK):
                tp = tps_p.tile([128, E], dt_mm, tag="tp")
                nc.tensor.transpose(
                    tp[:], gate_T[:, 128 * blk : 128 * (blk + 1)], ident[:]
                )
                nc.vector.tensor_copy(gate_sb[:, E * blk : E * (blk + 1)], tp[:])

            y_sb = y_p.tile([128, B], f32, tag="y")

            for e in range(E):
                w1_sb = w1_p.tile([128, H], dt_mm, tag="w1")
                nc.sync.dma_start(w1_sb[:], w1_d[tl, e])
                w2_sb = w2_p.tile([128, H], dt_mm, tag="w2")
                nc.sync.dma_start(
                    w2_sb[:].rearrange("p (hk d) -> p hk d", hk=4),
                    w2_d[tl, e].rearrange("(hk p) d -> p hk d", p=128),
                )
                b1_sb = small_p.tile([128, 4], f32, tag="b1")
                nc.sync.dma_start(b1_sb[:], b1_d[tl, e])

                # ---- h_T = gelu(W1slice^T @ xT + b1), laid out [128, (hb b)] ----
                h_sb = h_p.tile([128, 4 * B], dt_mm, tag="h")
                for hb in range(4):
                    for cc in range(2):  # two 1024-wide psum drains per hb
                        hps = hps_p.tile([128, 1024], f32, tag="hps")
                        for half in range(2):
                            c = 2 * cc + half
                            nc.tensor.matmul(
                                hps[:, 512 * half : 512 * (half + 1)],
                                w1_sb[:, 128 * hb : 128 * (hb + 1)],
                                xT[:, 512 * c : 512 * (c + 1)],
                                start=True, stop=True,
                            )
                        nc.scalar.activation(
                            h_sb[:, B * hb + 1024 * cc : B * hb + 1024 * (cc + 1)],
                            hps[:], AF.Gelu, bias=b1_sb[:, hb : hb + 1],
                        )

                # ---- expert out per 128-block, gated accumulate into y ----
                for g in range(4):  # groups of 4 blocks -> batched adds
                    if e > 0:
                        tmp = tmp_p.tile([128, 512], f32, tag="tmp")
                    else:
                        tmp = None
                    for j in range(4):
                        blk = 4 * g + j
                        yps = sps_p.tile([128, 128], f32, tag="sp")
                        for hk in range(4):
                            nc.tensor.matmul(
                                yps[:],
                                h_sb[:, B * hk + 128 * blk : B * hk + 128 * (blk + 1)],
                                w2_sb[:, 128 * hk : 128 * (hk + 1)],
                                start=(hk == 0), stop=(hk == 3),
                            )
                        gcol = gate_sb[:, E * blk + e : E * blk + e + 1]
                        if e == 0:
                            nc.vector.tensor_scalar(
                                y_sb[:, 512 * g + 128 * j : 512 * g + 128 * (j + 1)],
                                yps[:], gcol, None, bass.mybir.AluOpType.mult,
                            )
                        else:
                            nc.vector.tensor_scalar(
                                tmp[:, 128 * j : 128 * (j + 1)],
                                yps[:], gcol, None, bass.mybir.AluOpType.mult,
                            )
                    if e > 0:
                        nc.vector.tensor_add(
                            y_sb[:, 512 * g : 512 * (g + 1)],
                            y_sb[:, 512 * g : 512 * (g + 1)],
                            tmp[:],
                        )

            nc.sync.dma_start(
                y_d[:, tl, :].rearrange("(blk p) d -> p blk d", p=128),
                y_sb[:].rearrange("p (blk d) -> p blk d", blk=NBLK),
            )

    nc.compile()
    return nc


def get_program(dt_mm_name: str = "bfloat16"):
    key = ("nc", dt_mm_name)
    if key not in _CACHE:
        _install_ntff_hook()
        _CACHE[key] = _build(dt_mm_name)
    return _CACHE[key]


def _np_dt(dt_mm_name):
    if dt_mm_name == "bfloat16":
        import ml_dtypes

        return ml_dtypes.bfloat16
    return np.float32


def make_in_maps(x, W1, b1, W2, b2, gate_w_infer, gate_b_infer, dt_mm_name="bfloat16"):
    c = np.ascontiguousarray
    ndt = _np_dt(dt_mm_name)
    x = np.asarray(x, np.float32)
    W1 = np.asarray(W1, np.float32)
    b1 = np.asarray(b1, np.float32)
    W2 = np.asarray(W2, np.float32)
    gw = np.asarray(gate_w_infer, np.float32)
    gb = np.asarray(gate_b_infer, np.float32)
    ident = np.eye(E, dtype=np.float32)
    maps = []
    for i in range(N_CORES):
        s = slice(T_LOC * i, T_LOC * (i + 1))
        # xT[t, d, b] pre-transposed; b1 as [t, e, h%128, h//128]
        xTi = np.transpose(x[:, s, :], (1, 2, 0))
        b1i = np.transpose(b1[s].reshape(T_LOC, E, 4, 128), (0, 1, 3, 2))
        maps.append(
            {
                "xT": c(xTi.astype(ndt)),
                "w1": c(W1[s].astype(ndt)),
                "b1t": c(b1i),
                "w2": c(W2[s].astype(ndt)),
                "gw": c(gw[s].astype(ndt)),
                "gb": c(gb[s]),
                "ident": ident.astype(ndt),
            }
        )
    return maps


def kernel(x, W1, b1, W2, b2, gate_w_infer, gate_b_infer):
    from concourse.bass_utils import run_bass_kernel_spmd

    dt_mm_name = "bfloat16"
    nc = get_program(dt_mm_name)
    maps = make_in_maps(x, W1, b1, W2, b2, gate_w_infer, gate_b_infer, dt_mm_name)
    res = run_bass_kernel_spmd(nc, maps, list(range(N_CORES)))
    y = np.concatenate([res.results[i]["y"] for i in range(N_CORES)], axis=1)
    b2 = np.asarray(b2, np.float32)
    if np.any(b2):
        # b2 is all-zero for this problem's setup_inputs; handled host-side
        # for generality since the device kernel omits the b2 term.
        xf = np.asarray(x, np.float32)
        gate = np.einsum("btd,tde->bte", xf, np.asarray(gate_w_infer, np.float32))
        gate = np.maximum(gate + np.asarray(gate_b_infer, np.float32), 0.0)
        y = y + np.einsum("bte,ted->btd", gate, b2)
    return y, np.asarray(0.0, dtype=np.float32)


# revision 15
# speedup vs baseline: 1.1836x; 1.0180x over previous
"""Per-token sparse MoE kernel for Trainium2 (8 NeuronCores, Bass/Tile).

Problem: y[b,t,:] = sum_e relu(x[b,t]@gw[t])[e] * (gelu(x[b,t]@W1[t,e]+b1)@W2[t,e]+b2)
Shapes: x[2048,16,128], W1[16,4,128,512], W2[16,4,512,128], gates[16,128,4].

Sharding: the t dimension (16) is split across the 8 cores (2 t-values per
core). That makes the problem embarrassingly parallel (no collectives) and
each core only loads its own 1/8 of the weights (~2.1 MB in bf16) instead
of the full 33 MB, so the kernel is compute-bound rather than HBM-bound.

Host-side marshalling (inside kernel(), part of sharding): inputs are
sliced per-core, cast to the matmul dtype, and x is pre-transposed to
xT[t, d, b] so the device program needs no transpose/cast machinery for
its inputs.

Per-core device dataflow, per t:
  gate_T [E,B] = relu(gw^T @ xT)  (PE, gw stationary; ACT relu w/ bias)
  gate    [B,E]  by PE-transposing gate_T back (per 128-column block)
  h_T [H,B] = W1-slice^T @ xT     (PE, W1 stationary, 16 matmuls N=512)
  h = gelu(h_T + b1)              (ACT, exact-erf Gelu, per-partition bias)
  expert psum [Bblk,D] = h-block^T @ W2-block (PE, 4 accumulating matmuls)
  y += gate[:,e] * psum           (DVE tensor_scalar + batched adds)

b2 is all-zero in this problem; a host-side numpy correction covers the
general case.
"""

import contextlib
import ctypes
import sys
import types

import numpy as np

B, T, D, E, H = 2048, 16, 128, 4, 512
N_CORES = 8
T_LOC = T // N_CORES  # 2 t-values per core
NBLK = B // 128       # 16 b-blocks of 128
NCH = B // 512        # 4 b-chunks of 512 (matmul moving-operand max)

_CACHE: dict = {}


def _install_ntff_hook():
    """Provide antenv.axon_hooks (absent in this image) so that
    run_bass_kernel_spmd(trace=True) can capture NTFF profiles."""
    if "antenv.axon_hooks" in sys.modules:
        return
    try:
        lib = ctypes.CDLL("/opt/axon/libaxon_pjrt.so")
        if not hasattr(lib, "axon_start_nrt_profile"):
            hook = None
        else:
            lib.axon_start_nrt_profile.argtypes = [
                ctypes.POINTER(ctypes.c_int64),
                ctypes.c_size_t,
            ]
            lib.axon_start_nrt_profile.restype = ctypes.c_int64
            lib.axon_stop_nrt_profile.argtypes = [ctypes.c_char_p]
            lib.axon_stop_nrt_profile.restype = ctypes.c_int64

            @contextlib.contextmanager
            def hook(output_dir, device_ids):
                import jax

                jax.devices()
                if device_ids:
                    ids = (ctypes.c_int64 * len(device_ids))(*device_ids)
                    rc = lib.axon_start_nrt_profile(ids, len(device_ids))
                else:
                    rc = lib.axon_start_nrt_profile(None, 0)
                if rc != 0:
                    raise RuntimeError(f"axon_start_nrt_profile rc={rc}")
                try:
                    yield
                finally:
                    lib.axon_stop_nrt_profile(str(output_dir).encode())

        m = types.ModuleType("antenv.axon_hooks")
        m.get_axon_ntff_profile_hook = lambda: hook
        m.set_axon_ntff_profile_hook = lambda h: None
        sys.modules["antenv.axon_hooks"] = m
        import antenv

        antenv.axon_hooks = m
    except OSError:
        pass


def _build(dt_mm_name: str = "bfloat16"):
    """Build and compile the per-core Bass program. Same program on all cores.

    dt_mm_name selects the matmul-operand dtype (host pre-casts inputs):
      float32  — exact, but every matmul is a 2-pass HI/LO pair (slow)
      float32r — single-pass fp22-truncated reads (~2.6e-4 rel err)
      bfloat16 — single-pass + fast weight load (~4e-3 rel err)
    PSUM accumulation is fp32 in all cases.
    """
    import concourse.bass as bass
    import concourse.tile as tile
    from concourse import bacc, mybir

    dt_mm = getattr(mybir.dt, dt_mm_name)
    f32 = mybir.dt.float32
    AF = mybir.ActivationFunctionType

    nc = bacc.Bacc("TRN2", target_bir_lowering=False, debug=False, num_devices=N_CORES)

    xT_d = nc.dram_tensor("xT", [T_LOC, D, B], dt_mm, kind="ExternalInput").ap()
    w1_d = nc.dram_tensor("w1", [T_LOC, E, D, H], dt_mm, kind="ExternalInput").ap()
    b1_d = nc.dram_tensor("b1t", [T_LOC, E, 128, 4], f32, kind="ExternalInput").ap()
    w2_d = nc.dram_tensor("w2", [T_LOC, E, H, D], dt_mm, kind="ExternalInput").ap()
    gw_d = nc.dram_tensor("gw", [T_LOC, D, E], dt_mm, kind="ExternalInput").ap()
    gb_d = nc.dram_tensor("gb", [T_LOC, E], f32, kind="ExternalInput").ap()
    id_d = nc.dram_tensor("ident", [E, E], dt_mm, kind="ExternalInput").ap()
    y_d = nc.dram_tensor("y", [B, T_LOC, D], f32, kind="ExternalOutput").ap()

    with tile.TileContext(nc) as tc, contextlib.ExitStack() as ctx:
        ep = ctx.enter_context
        # SBUF pools
        const_p = ep(tc.tile_pool(name="const", bufs=1))
        xT_p = ep(tc.tile_pool(name="xT", bufs=2))
        h_p = ep(tc.tile_pool(name="h", bufs=2))
        w1_p = ep(tc.tile_pool(name="w1", bufs=3))
        w2_p = ep(tc.tile_pool(name="w2", bufs=3))
        y_p = ep(tc.tile_pool(name="y", bufs=2))
        tmp_p = ep(tc.tile_pool(name="tmp", bufs=4))
        gt_p = ep(tc.tile_pool(name="gt", bufs=2))
        small_p = ep(tc.tile_pool(name="small", bufs=4))
        # PSUM pools: hps 2x2 + sp 2 + tp 2 = 8 banks (gate psum shares "sp")
        hps_p = ep(tc.tile_pool(name="hps", bufs=2, space="PSUM"))
        sps_p = ep(tc.tile_pool(name="sps", bufs=2, space="PSUM"))
        tps_p = ep(tc.tile_pool(name="tps", bufs=2, space="PSUM"))

        ident = const_p.tile([E, E], dt_mm)
        nc.sync.dma_start(ident[:], id_d[:])

        for tl in range(T_LOC):
            # ---- xT [d, b]: chunked contiguous loads (pre-transposed on host)
            xT = xT_p.tile([128, B], dt_mm, tag="xT")
            for c in range(NCH):
                nc.sync.dma_start(
                    xT[:, 512 * c : 512 * (c + 1)], xT_d[tl, :, 512 * c : 512 * (c + 1)]
                )

            def emit_mm1(e):
                w1_sb = w1_p.tile([128, H], dt_mm, tag="w1", name=f"w1_{tl}_{e}")
                nc.sync.dma_start(w1_sb[:], w1_d[tl, e])
                b1_sb = small_p.tile([128, 4], f32, tag="b1", name=f"b1_{tl}_{e}")
                nc.sync.dma_start(b1_sb[:], b1_d[tl, e])
                # h_T = gelu(W1slice^T @ xT + b1), laid out [128, (hb b)]
                h_sb = h_p.tile([128, 4 * B], dt_mm, tag="h", name=f"h_{tl}_{e}")
                for hb in range(4):
                    for cc in range(2):  # two 1024-wide psum drains per hb
                        hps = hps_p.tile([128, 1024], f32, tag="hps", name=f"hps_{tl}_{e}_{hb}_{cc}")
                        for half in range(2):
                            c = 2 * cc + half
                            nc.tensor.matmul(
                                hps[:, 512 * half : 512 * (half + 1)],
                                w1_sb[:, 128 * hb : 128 * (hb + 1)],
                                xT[:, 512 * c : 512 * (c + 1)],
                                start=True, stop=True,
                            )
                        nc.scalar.activation(
                            h_sb[:, B * hb + 1024 * cc : B * hb + 1024 * (cc + 1)],
                            hps[:], AF.Gelu, bias=b1_sb[:, hb : hb + 1],
                        )
                return h_sb

            def emit_mm2(e, h_sb):
                w2_sb = w2_p.tile([128, H], dt_mm, tag="w2", name=f"w2_{tl}_{e}")
                nc.sync.dma_start(
                    w2_sb[:].rearrange("p (hk d) -> p hk d", hk=4),
                    w2_d[tl, e].rearrange("(hk p) d -> p hk d", p=128),
                )
                # expert out per 128-block, gated accumulate into y
                for g in range(4):  # groups of 4 blocks -> batched adds
                    if e > 0:
                        tmp = tmp_p.tile([128, 512], f32, tag="tmp", name=f"tmp_{tl}_{e}_{g}")
                    else:
                        tmp = None
                    for j in range(4):
                        blk = 4 * g + j
                        yps = sps_p.tile([128, 128], f32, tag="sp", name=f"yps_{tl}_{e}_{blk}")
                        for hk in range(4):
                            nc.tensor.matmul(
                                yps[:],
                                h_sb[:, B * hk + 128 * blk : B * hk + 128 * (blk + 1)],
                                w2_sb[:, 128 * hk : 128 * (hk + 1)],
                                start=(hk == 0), stop=(hk == 3),
                            )
                        gcol = gate_sb[:, E * blk + e : E * blk + e + 1]
                        if e == 0:
                            nc.vector.tensor_scalar(
                                y_sb[:, 512 * g + 128 * j : 512 * g + 128 * (j + 1)],
                                yps[:], gcol, None, bass.mybir.AluOpType.mult,
                            )
                        else:
                            nc.vector.tensor_scalar(
                                tmp[:, 128 * j : 128 * (j + 1)],
                                yps[:], gcol, None, bass.mybir.AluOpType.mult,
                            )
                    if e > 0:
                        nc.gpsimd.tensor_add(
                            y_sb[:, 512 * g : 512 * (g + 1)],
                            y_sb[:, 512 * g : 512 * (g + 1)],
                            tmp[:],
                        )

            y_sb = y_p.tile([128, B], f32, tag="y")

            # mm1(e0) first so ACT's gelu stream starts as early as possible
            h0 = emit_mm1(0)

            # ---- gate_T [E, B] = relu(gw^T @ xT + gb)  (relu on DVE) ----
            gw_sb = small_p.tile([128, E], dt_mm, tag="gw")
            nc.sync.dma_start(gw_sb[:], gw_d[tl])
            gb_sb = small_p.tile([E, 1], f32, tag="gb")
            nc.sync.dma_start(gb_sb[:], gb_d[tl])
            gate_T = gt_p.tile([E, B], dt_mm, tag="gateT")
            for c in range(NCH):
                gps = sps_p.tile([E, 512], f32, tag="sp")
                nc.tensor.matmul(
                    gps[:], gw_sb[:], xT[:, 512 * c : 512 * (c + 1)],
                    start=True, stop=True,
                )
                nc.vector.tensor_scalar(
                    gate_T[:, 512 * c : 512 * (c + 1)], gps[:],
                    gb_sb[:, 0:1], 0.0,
                    bass.mybir.AluOpType.add, bass.mybir.AluOpType.max,
                )
            # gate [B, E] per-block by PE-transposing gate_T back
            gate_sb = gt_p.tile([128, E * NBLK], f32, tag="gate")
            for blk in range(NBLK):
                tp = tps_p.tile([128, E], dt_mm, tag="tp")
                nc.tensor.transpose(
                    tp[:], gate_T[:, 128 * blk : 128 * (blk + 1)], ident[:]
                )
                nc.vector.tensor_copy(gate_sb[:, E * blk : E * (blk + 1)], tp[:])

            emit_mm2(0, h0)
            for e in range(1, E):
                h_sb = emit_mm1(e)
                emit_mm2(e, h_sb)

            nc.sync.dma_start(
                y_d[:, tl, :].rearrange("(blk p) d -> p blk d", p=128),
                y_sb[:].rearrange("p (blk d) -> p blk d", blk=NBLK),
            )

    nc.compile()
    return nc


def get_program(dt_mm_name: str = "bfloat16"):
    key = ("nc", dt_mm_name)
    if key not in _CACHE:
        _install_ntff_hook()
        _CACHE[key] = _build(dt_mm_name)
    return _CACHE[key]


def _np_dt(dt_mm_name):
    if dt_mm_name == "bfloat16":
        import ml_dtypes

        return ml_dtypes.bfloat16
    return np.float32


def make_in_maps(x, W1, b1, W2, b2, gate_w_infer, gate_b_infer, dt_mm_name="bfloat16"):
    c = np.ascontiguousarray
    ndt = _np_dt(dt_mm_name)
    x = np.asarray(x, np.float32)
    W1 = np.asarray(W1, np.float32)
    b1 = np.asarray(b1, np.float32)
    W2 = np.asarray(W2, np.float32)
    gw = np.asarray(gate_w_infer, np.float32)
    gb = np.asarray(gate_b_infer, np.float32)
    ident = np.eye(E, dtype=np.float32)
    maps = []
    for i in range(N_CORES):
        s = slice(T_LOC * i, T_LOC * (i + 1))
        # xT[t, d, b] pre-transposed; b1 as [t, e, h%128, h//128]
        xTi = np.transpose(x[:, s, :], (1, 2, 0))
        b1i = np.transpose(b1[s].reshape(T_LOC, E, 4, 128), (0, 1, 3, 2))
        maps.append(
            {
                "xT": c(xTi.astype(ndt)),
                "w1": c(W1[s].astype(ndt)),
                "b1t": c(b1i),
                "w2": c(W2[s].astype(ndt)),
                "gw": c(gw[s].astype(ndt)),
                "gb": c(gb[s]),
                "ident": ident.astype(ndt),
            }
        )
    return maps


def kernel(x, W1, b1, W2, b2, gate_w_infer, gate_b_infer):
    from concourse.bass_utils import run_bass_kernel_spmd

    dt_mm_name = "bfloat16"
    nc = get_program(dt_mm_name)
    maps = make_in_maps(x, W1, b1, W2, b2, gate_w_infer, gate_b_infer, dt_mm_name)
    res = run_bass_kernel_spmd(nc, maps, list(range(N_CORES)))
    y = np.concatenate([res.results[i]["y"] for i in range(N_CORES)], axis=1)
    b2 = np.asarray(b2, np.float32)
    if np.any(b2):
        # b2 is all-zero for this problem's setup_inputs; handled host-side
        # for generality since the device kernel omits the b2 term.
        xf = np.asarray(x, np.float32)
        gate = np.einsum("btd,tde->bte", xf, np.asarray(gate_w_infer, np.float32))
        gate = np.maximum(gate + np.asarray(gate_b_infer, np.float32), 0.0)
        y = y + np.einsum("bte,ted->btd", gate, b2)
    return y, np.asarray(0.0, dtype=np.float32)


# revision 16
# speedup vs baseline: 1.2108x; 1.0230x over previous
"""Per-token sparse MoE kernel for Trainium2 (8 NeuronCores, Bass/Tile).

Problem: y[b,t,:] = sum_e relu(x[b,t]@gw[t])[e] * (gelu(x[b,t]@W1[t,e]+b1)@W2[t,e]+b2)
Shapes: x[2048,16,128], W1[16,4,128,512], W2[16,4,512,128], gates[16,128,4].

Sharding: the t dimension (16) is split across the 8 cores (2 t-values per
core). That makes the problem embarrassingly parallel (no collectives) and
each core only loads its own 1/8 of the weights (~2.1 MB in bf16) instead
of the full 33 MB, so the kernel is compute-bound rather than HBM-bound.

Host-side marshalling (inside kernel(), part of sharding): inputs are
sliced per-core, cast to the matmul dtype, and x is pre-transposed to
xT[t, d, b] so the device program needs no transpose/cast machinery for
its inputs.

Per-core device dataflow, per t:
  gate_T [E,B] = relu(gw^T @ xT)  (PE, gw stationary; ACT relu w/ bias)
  gate    [B,E]  by PE-transposing gate_T back (per 128-column block)
  h_T [H,B] = W1-slice^T @ xT     (PE, W1 stationary, 16 matmuls N=512)
  h = gelu(h_T + b1)              (ACT, exact-erf Gelu, per-partition bias)
  expert psum [Bblk,D] = h-block^T @ W2-block (PE, 4 accumulating matmuls)
  y += gate[:,e] * psum           (DVE tensor_scalar + batched adds)

b2 is all-zero in this problem; a host-side numpy correction covers the
general case.
"""

import contextlib
import ctypes
import sys
import types

import numpy as np

B, T, D, E, H = 2048, 16, 128, 4, 512
N_CORES = 8
T_LOC = T // N_CORES  # 2 t-values per core
NBLK = B // 128       # 16 b-blocks of 128
NCH = B // 512        # 4 b-chunks of 512 (matmul moving-operand max)

_CACHE: dict = {}


def _install_ntff_hook():
    """Provide antenv.axon_hooks (absent in this image) so that
    run_bass_kernel_spmd(trace=True) can capture NTFF profiles."""
    if "antenv.axon_hooks" in sys.modules:
        return
    try:
        lib = ctypes.CDLL("/opt/axon/libaxon_pjrt.so")
        if not hasattr(lib, "axon_start_nrt_profile"):
            hook = None
        else:
            lib.axon_start_nrt_profile.argtypes = [
                ctypes.POINTER(ctypes.c_int64),
                ctypes.c_size_t,
            ]
            lib.axon_start_nrt_profile.restype = ctypes.c_int64
            lib.axon_stop_nrt_profile.argtypes = [ctypes.c_char_p]
            lib.axon_stop_nrt_profile.restype = ctypes.c_int64

            @contextlib.contextmanager
            def hook(output_dir, device_ids):
                import jax

                jax.devices()
                if device_ids:
                    ids = (ctypes.c_int64 * len(device_ids))(*device_ids)
                    rc = lib.axon_start_nrt_profile(ids, len(device_ids))
                else:
                    rc = lib.axon_start_nrt_profile(None, 0)
                if rc != 0:
                    raise RuntimeError(f"axon_start_nrt_profile rc={rc}")
                try:
                    yield
                finally:
                    lib.axon_stop_nrt_profile(str(output_dir).encode())

        m = types.ModuleType("antenv.axon_hooks")
        m.get_axon_ntff_profile_hook = lambda: hook
        m.set_axon_ntff_profile_hook = lambda h: None
        sys.modules["antenv.axon_hooks"] = m
        import antenv

        antenv.axon_hooks = m
    except OSError:
        pass


def _build(dt_mm_name: str = "bfloat16"):
    """Build and compile the per-core Bass program. Same program on all cores.

    dt_mm_name selects the matmul-operand dtype (host pre-casts inputs):
      float32  — exact, but every matmul is a 2-pass HI/LO pair (slow)
      float32r — single-pass fp22-truncated reads (~2.6e-4 rel err)
      bfloat16 — single-pass + fast weight load (~4e-3 rel err)
    PSUM accumulation is fp32 in all cases.
    """
    import concourse.bass as bass
    import concourse.tile as tile
    from concourse import bacc, mybir

    dt_mm = getattr(mybir.dt, dt_mm_name)
    f32 = mybir.dt.float32
    AF = mybir.ActivationFunctionType

    nc = bacc.Bacc("TRN2", target_bir_lowering=False, debug=False, num_devices=N_CORES)

    xT_d = nc.dram_tensor("xT", [T_LOC, D, B], dt_mm, kind="ExternalInput").ap()
    w1_d = nc.dram_tensor("w1", [T_LOC, E, D, H], dt_mm, kind="ExternalInput").ap()
    b1_d = nc.dram_tensor("b1t", [T_LOC, E, 128, 4], f32, kind="ExternalInput").ap()
    w2_d = nc.dram_tensor("w2", [T_LOC, E, H, D], dt_mm, kind="ExternalInput").ap()
    gw_d = nc.dram_tensor("gw", [T_LOC, D, E], dt_mm, kind="ExternalInput").ap()
    gb_d = nc.dram_tensor("gb", [T_LOC, E], f32, kind="ExternalInput").ap()
    id_d = nc.dram_tensor("ident", [E, E], dt_mm, kind="ExternalInput").ap()
    y_d = nc.dram_tensor("y", [B, T_LOC, D], f32, kind="ExternalOutput").ap()

    with tile.TileContext(nc) as tc, contextlib.ExitStack() as ctx:
        ep = ctx.enter_context
        # SBUF pools
        const_p = ep(tc.tile_pool(name="const", bufs=1))
        xT_p = ep(tc.tile_pool(name="xT", bufs=2))
        h_p = ep(tc.tile_pool(name="h", bufs=2))
        w1_p = ep(tc.tile_pool(name="w1", bufs=3))
        w2_p = ep(tc.tile_pool(name="w2", bufs=3))
        y_p = ep(tc.tile_pool(name="y", bufs=2))
        tmp_p = ep(tc.tile_pool(name="tmp", bufs=4))
        gt_p = ep(tc.tile_pool(name="gt", bufs=2))
        small_p = ep(tc.tile_pool(name="small", bufs=4))
        # PSUM pools: hps 3x2 + sp 2 = 8 banks (gate/transpose psums share "sp")
        hps_p = ep(tc.tile_pool(name="hps", bufs=3, space="PSUM"))
        sps_p = ep(tc.tile_pool(name="sps", bufs=2, space="PSUM"))

        ident = const_p.tile([E, E], dt_mm)
        nc.scalar.dma_start(ident[:], id_d[:])

        for tl in range(T_LOC):
            # ---- xT [d, b]: one contiguous load (pre-transposed on host)
            xT = xT_p.tile([128, B], dt_mm, tag="xT")
            nc.scalar.dma_start(xT[:], xT_d[tl])

            def emit_mm1(e):
                w1_sb = w1_p.tile([128, H], dt_mm, tag="w1", name=f"w1_{tl}_{e}")
                nc.sync.dma_start(w1_sb[:], w1_d[tl, e])
                b1_sb = small_p.tile([128, 4], f32, tag="b1", name=f"b1_{tl}_{e}")
                nc.sync.dma_start(b1_sb[:], b1_d[tl, e])
                # h_T = gelu(W1slice^T @ xT + b1), laid out [128, (hb b)]
                h_sb = h_p.tile([128, 4 * B], dt_mm, tag="h", name=f"h_{tl}_{e}")
                for hb in range(4):
                    for cc in range(2):  # two 1024-wide psum drains per hb
                        hps = hps_p.tile([128, 1024], f32, tag="hps", name=f"hps_{tl}_{e}_{hb}_{cc}")
                        for half in range(2):
                            c = 2 * cc + half
                            nc.tensor.matmul(
                                hps[:, 512 * half : 512 * (half + 1)],
                                w1_sb[:, 128 * hb : 128 * (hb + 1)],
                                xT[:, 512 * c : 512 * (c + 1)],
                                start=True, stop=True,
                            )
                        nc.scalar.activation(
                            h_sb[:, B * hb + 1024 * cc : B * hb + 1024 * (cc + 1)],
                            hps[:], AF.Gelu, bias=b1_sb[:, hb : hb + 1],
                        )
                return h_sb

            def emit_mm2(e, h_sb):
                w2_sb = w2_p.tile([128, H], dt_mm, tag="w2", name=f"w2_{tl}_{e}")
                nc.sync.dma_start(
                    w2_sb[:].rearrange("p (hk d) -> p hk d", hk=4),
                    w2_d[tl, e].rearrange("(hk p) d -> p hk d", p=128),
                )
                # expert out per 128-block, gated accumulate into y
                for g in range(4):  # groups of 4 blocks -> batched adds
                    if e > 0:
                        tmp = tmp_p.tile([128, 512], f32, tag="tmp", name=f"tmp_{tl}_{e}_{g}")
                    else:
                        tmp = None
                    for j in range(4):
                        blk = 4 * g + j
                        yps = sps_p.tile([128, 128], f32, tag="sp", name=f"yps_{tl}_{e}_{blk}")
                        for hk in range(4):
                            nc.tensor.matmul(
                                yps[:],
                                h_sb[:, B * hk + 128 * blk : B * hk + 128 * (blk + 1)],
                                w2_sb[:, 128 * hk : 128 * (hk + 1)],
                                start=(hk == 0), stop=(hk == 3),
                            )
                        gcol = gate_sb[:, E * blk + e : E * blk + e + 1]
                        if e == 0:
                            nc.vector.tensor_scalar(
                                y_sb[:, 512 * g + 128 * j : 512 * g + 128 * (j + 1)],
                                yps[:], gcol, None, bass.mybir.AluOpType.mult,
                            )
                        else:
                            nc.vector.tensor_scalar(
                                tmp[:, 128 * j : 128 * (j + 1)],
                                yps[:], gcol, None, bass.mybir.AluOpType.mult,
                            )
                    if e > 0:
                        nc.gpsimd.tensor_add(
                            y_sb[:, 512 * g : 512 * (g + 1)],
                            y_sb[:, 512 * g : 512 * (g + 1)],
                            tmp[:],
                        )

            y_sb = y_p.tile([128, B], f32, tag="y")

            # mm1(e0) first so ACT's gelu stream starts as early as possible
            h0 = emit_mm1(0)

            # ---- gate_T [E, B] = relu(gw^T @ xT + gb)  (relu on DVE) ----
            gw_sb = small_p.tile([128, E], dt_mm, tag="gw")
            nc.scalar.dma_start(gw_sb[:], gw_d[tl])
            gb_sb = small_p.tile([E, 1], f32, tag="gb")
            nc.scalar.dma_start(gb_sb[:], gb_d[tl])
            gate_T = gt_p.tile([E, B], dt_mm, tag="gateT")
            for c in range(NCH):
                gps = sps_p.tile([E, 512], f32, tag="sp")
                nc.tensor.matmul(
                    gps[:], gw_sb[:], xT[:, 512 * c : 512 * (c + 1)],
                    start=True, stop=True,
                )
                nc.vector.tensor_scalar(
                    gate_T[:, 512 * c : 512 * (c + 1)], gps[:],
                    gb_sb[:, 0:1], 0.0,
                    bass.mybir.AluOpType.add, bass.mybir.AluOpType.max,
                )
            # gate [B, E] per-block by PE-transposing gate_T back
            gate_sb = gt_p.tile([128, E * NBLK], f32, tag="gate")
            for blk in range(NBLK):
                tp = sps_p.tile([128, E], dt_mm, tag="sp", name=f"tp_{tl}_{blk}")
                nc.tensor.transpose(
                    tp[:], gate_T[:, 128 * blk : 128 * (blk + 1)], ident[:]
                )
                nc.vector.tensor_copy(gate_sb[:, E * blk : E * (blk + 1)], tp[:])

            emit_mm2(0, h0)
            for e in range(1, E):
                h_sb = emit_mm1(e)
                emit_mm2(e, h_sb)

            nc.sync.dma_start(
                y_d[:, tl, :].rearrange("(blk p) d -> p blk d", p=128),
                y_sb[:].rearrange("p (blk d) -> p blk d", blk=NBLK),
            )

    nc.compile()
    return nc


def get_program(dt_mm_name: str = "bfloat16"):
    key = ("nc", dt_mm_name)
    if key not in _CACHE:
        _install_ntff_hook()
        _CACHE[key] = _build(dt_mm_name)
    return _CACHE[key]


def _np_dt(dt_mm_name):
    if dt_mm_name == "bfloat16":
        import ml_dtypes

        return ml_dtypes.bfloat16
    return np.float32


def make_in_maps(x, W1, b1, W2, b2, gate_w_infer, gate_b_infer, dt_mm_name="bfloat16"):
    c = np.ascontiguousarray
    ndt = _np_dt(dt_mm_name)
    x = np.asarray(x, np.float32)
    W1 = np.asarray(W1, np.float32)
    b1 = np.asarray(b1, np.float32)
    W2 = np.asarray(W2, np.float32)
    gw = np.asarray(gate_w_infer, np.float32)
    gb = np.asarray(gate_b_infer, np.float32)
    ident = np.eye(E, dtype=np.float32)
    maps = []
    for i in range(N_CORES):
        s = slice(T_LOC * i, T_LOC * (i + 1))
        # xT[t, d, b] pre-transposed; b1 as [t, e, h%128, h//128]
        xTi = np.transpose(x[:, s, :], (1, 2, 0))
        b1i = np.transpose(b1[s].reshape(T_LOC, E, 4, 128), (0, 1, 3, 2))
        maps.append(
            {
                "xT": c(xTi.astype(ndt)),
                "w1": c(W1[s].astype(ndt)),
                "b1t": c(b1i),
                "w2": c(W2[s].astype(ndt)),
                "gw": c(gw[s].astype(ndt)),
                "gb": c(gb[s]),
                "ident": ident.astype(ndt),
            }
        )
    return maps


def kernel(x, W1, b1, W2, b2, gate_w_infer, gate_b_infer):
    from concourse.bass_utils import run_bass_kernel_spmd

    dt_mm_name = "bfloat16"
    nc = get_program(dt_mm_name)
    maps = make_in_maps(x, W1, b1, W2, b2, gate_w_infer, gate_b_infer, dt_mm_name)
    res = run_bass_kernel_spmd(nc, maps, list(range(N_CORES)))
    y = np.concatenate([res.results[i]["y"] for i in range(N_CORES)], axis=1)
    b2 = np.asarray(b2, np.float32)
    if np.any(b2):
        # b2 is all-zero for this problem's setup_inputs; handled host-side
        # for generality since the device kernel omits the b2 term.
        xf = np.asarray(x, np.float32)
        gate = np.einsum("btd,tde->bte", xf, np.asarray(gate_w_infer, np.float32))
        gate = np.maximum(gate + np.asarray(gate_b_infer, np.float32), 0.0)
        y = y + np.einsum("bte,ted->btd", gate, b2)
    return y, np.asarray(0.0, dtype=np.float32)


# revision 17
# speedup vs baseline: 1.2534x; 1.0352x over previous
"""Per-token sparse MoE kernel for Trainium2 (8 NeuronCores, Bass/Tile).

Problem: y[b,t,:] = sum_e relu(x[b,t]@gw[t])[e] * (gelu(x[b,t]@W1[t,e]+b1)@W2[t,e]+b2)
Shapes: x[2048,16,128], W1[16,4,128,512], W2[16,4,512,128], gates[16,128,4].

Sharding: the t dimension (16) is split across the 8 cores (2 t-values per
core). That makes the problem embarrassingly parallel (no collectives) and
each core only loads its own 1/8 of the weights (~2.1 MB in bf16) instead
of the full 33 MB, so the kernel is compute-bound rather than HBM-bound.

Host-side marshalling (inside kernel(), part of sharding): inputs are
sliced per-core, cast to the matmul dtype, and x is pre-transposed to
xT[t, d, b] so the device program needs no transpose/cast machinery for
its inputs.

Per-core device dataflow, per t:
  gate_T [E,B] = relu(gw^T @ xT)  (PE, gw stationary; ACT relu w/ bias)
  gate    [B,E]  by PE-transposing gate_T back (per 128-column block)
  h_T [H,B] = W1-slice^T @ xT     (PE, W1 stationary, 16 matmuls N=512)
  h = gelu(h_T + b1)              (ACT, exact-erf Gelu, per-partition bias)
  expert psum [Bblk,D] = h-block^T @ W2-block (PE, 4 accumulating matmuls)
  y += gate[:,e] * psum           (DVE tensor_scalar + batched adds)

b2 is all-zero in this problem; a host-side numpy correction covers the
general case.
"""

import contextlib
import ctypes
import sys
import types

import numpy as np

B, T, D, E, H = 2048, 16, 128, 4, 512
N_CORES = 8
T_LOC = T // N_CORES  # 2 t-values per core
NBLK = B // 128       # 16 b-blocks of 128
NCH = B // 512        # 4 b-chunks of 512 (matmul moving-operand max)

_CACHE: dict = {}


def _install_ntff_hook():
    """Provide antenv.axon_hooks (absent in this image) so that
    run_bass_kernel_spmd(trace=True) can capture NTFF profiles."""
    if "antenv.axon_hooks" in sys.modules:
        return
    try:
        lib = ctypes.CDLL("/opt/axon/libaxon_pjrt.so")
        if not hasattr(lib, "axon_start_nrt_profile"):
            hook = None
        else:
            lib.axon_start_nrt_profile.argtypes = [
                ctypes.POINTER(ctypes.c_int64),
                ctypes.c_size_t,
            ]
            lib.axon_start_nrt_profile.restype = ctypes.c_int64
            lib.axon_stop_nrt_profile.argtypes = [ctypes.c_char_p]
            lib.axon_stop_nrt_profile.restype = ctypes.c_int64

            @contextlib.contextmanager
            def hook(output_dir, device_ids):
                import jax

                jax.devices()
                if device_ids:
                    ids = (ctypes.c_int64 * len(device_ids))(*device_ids)
                    rc = lib.axon_start_nrt_profile(ids, len(device_ids))
                else:
                    rc = lib.axon_start_nrt_profile(None, 0)
                if rc != 0:
                    raise RuntimeError(f"axon_start_nrt_profile rc={rc}")
                try:
                    yield
                finally:
                    lib.axon_stop_nrt_profile(str(output_dir).encode())

        m = types.ModuleType("antenv.axon_hooks")
        m.get_axon_ntff_profile_hook = lambda: hook
        m.set_axon_ntff_profile_hook = lambda h: None
        sys.modules["antenv.axon_hooks"] = m
        import antenv

        antenv.axon_hooks = m
    except OSError:
        pass


def _build(dt_mm_name: str = "bfloat16"):
    """Build and compile the per-core Bass program. Same program on all cores.

    dt_mm_name selects the matmul-operand dtype (host pre-casts inputs):
      float32  — exact, but every matmul is a 2-pass HI/LO pair (slow)
      float32r — single-pass fp22-truncated reads (~2.6e-4 rel err)
      bfloat16 — single-pass + fast weight load (~4e-3 rel err)
    PSUM accumulation is fp32 in all cases.
    """
    import concourse.bass as bass
    import concourse.tile as tile
    from concourse import bacc, mybir

    dt_mm = getattr(mybir.dt, dt_mm_name)
    f32 = mybir.dt.float32
    AF = mybir.ActivationFunctionType

    nc = bacc.Bacc("TRN2", target_bir_lowering=False, debug=False, num_devices=N_CORES)

    xT_d = nc.dram_tensor("xT", [T_LOC, D, B], dt_mm, kind="ExternalInput").ap()
    w1_d = nc.dram_tensor("w1", [T_LOC, E, D, H], dt_mm, kind="ExternalInput").ap()
    b1_d = nc.dram_tensor("b1t", [T_LOC, E, 128, 4], f32, kind="ExternalInput").ap()
    w2_d = nc.dram_tensor("w2", [T_LOC, E, H, D], dt_mm, kind="ExternalInput").ap()
    gw_d = nc.dram_tensor("gw", [T_LOC, D, E], dt_mm, kind="ExternalInput").ap()
    gb_d = nc.dram_tensor("gb", [T_LOC, E], f32, kind="ExternalInput").ap()
    id_d = nc.dram_tensor("ident", [E, E], dt_mm, kind="ExternalInput").ap()
    y_d = nc.dram_tensor("y", [B, T_LOC, D], f32, kind="ExternalOutput").ap()

    with tile.TileContext(nc) as tc, contextlib.ExitStack() as ctx:
        ep = ctx.enter_context
        # SBUF pools
        const_p = ep(tc.tile_pool(name="const", bufs=1))
        xT_p = ep(tc.tile_pool(name="xT", bufs=2))
        h_p = ep(tc.tile_pool(name="h", bufs=2))
        w1_p = ep(tc.tile_pool(name="w1", bufs=3))
        w2_p = ep(tc.tile_pool(name="w2", bufs=3))
        y_p = ep(tc.tile_pool(name="y", bufs=2))
        tmp_p = ep(tc.tile_pool(name="tmp", bufs=4))
        gt_p = ep(tc.tile_pool(name="gt", bufs=2))
        small_p = ep(tc.tile_pool(name="small", bufs=4))
        # PSUM pools: hps 3x2 + sp 2 = 8 banks (gate/transpose psums share "sp")
        hps_p = ep(tc.tile_pool(name="hps", bufs=3, space="PSUM"))
        sps_p = ep(tc.tile_pool(name="sps", bufs=2, space="PSUM"))

        ident = const_p.tile([E, E], dt_mm)
        nc.gpsimd.dma_start(ident[:], id_d[:])

        for tl in range(T_LOC):
            # ---- first-expert weights, then xT chunks (Sync issues in order;
            # the first mm1 matmul needs w1 + xT chunk 0 only)
            w1_e0 = w1_p.tile([128, H], dt_mm, tag="w1", name=f"w1_{tl}_0")
            nc.sync.dma_start(w1_e0[:], w1_d[tl, 0])
            b1_e0 = small_p.tile([128, 4], f32, tag="b1", name=f"b1_{tl}_0")
            nc.sync.dma_start(b1_e0[:], b1_d[tl, 0])
            xT = xT_p.tile([128, B], dt_mm, tag="xT")
            for c in range(NCH):
                nc.sync.dma_start(
                    xT[:, 512 * c : 512 * (c + 1)], xT_d[tl, :, 512 * c : 512 * (c + 1)]
                )

            def emit_mm1(e):
                if e == 0:
                    w1_sb, b1_sb = w1_e0, b1_e0
                else:
                    w1_sb = w1_p.tile([128, H], dt_mm, tag="w1", name=f"w1_{tl}_{e}")
                    nc.sync.dma_start(w1_sb[:], w1_d[tl, e])
                    b1_sb = small_p.tile([128, 4], f32, tag="b1", name=f"b1_{tl}_{e}")
                    nc.sync.dma_start(b1_sb[:], b1_d[tl, e])
                # h_T = gelu(W1slice^T @ xT + b1), laid out [128, (hb b)]
                h_sb = h_p.tile([128, 4 * B], dt_mm, tag="h", name=f"h_{tl}_{e}")
                for hb in range(4):
                    for cc in range(2):  # two 1024-wide psum drains per hb
                        hps = hps_p.tile([128, 1024], f32, tag="hps", name=f"hps_{tl}_{e}_{hb}_{cc}")
                        for half in range(2):
                            c = 2 * cc + half
                            nc.tensor.matmul(
                                hps[:, 512 * half : 512 * (half + 1)],
                                w1_sb[:, 128 * hb : 128 * (hb + 1)],
                                xT[:, 512 * c : 512 * (c + 1)],
                                start=True, stop=True,
                            )
                        nc.scalar.activation(
                            h_sb[:, B * hb + 1024 * cc : B * hb + 1024 * (cc + 1)],
                            hps[:], AF.Gelu, bias=b1_sb[:, hb : hb + 1],
                        )
                return h_sb

            def emit_mm2(e, h_sb):
                w2_sb = w2_p.tile([128, H], dt_mm, tag="w2", name=f"w2_{tl}_{e}")
                nc.sync.dma_start(
                    w2_sb[:].rearrange("p (hk d) -> p hk d", hk=4),
                    w2_d[tl, e].rearrange("(hk p) d -> p hk d", p=128),
                )
                # expert out per 128-block, gated accumulate into y
                for g in range(4):  # groups of 4 blocks -> batched adds
                    if e > 0:
                        tmp = tmp_p.tile([128, 512], f32, tag="tmp", name=f"tmp_{tl}_{e}_{g}")
                    else:
                        tmp = None
                    for j in range(4):
                        blk = 4 * g + j
                        yps = sps_p.tile([128, 128], f32, tag="sp", name=f"yps_{tl}_{e}_{blk}")
                        for hk in range(4):
                            nc.tensor.matmul(
                                yps[:],
                                h_sb[:, B * hk + 128 * blk : B * hk + 128 * (blk + 1)],
                                w2_sb[:, 128 * hk : 128 * (hk + 1)],
                                start=(hk == 0), stop=(hk == 3),
                            )
                        gcol = gate_sb[:, E * blk + e : E * blk + e + 1]
                        if e == 0:
                            nc.vector.tensor_scalar(
                                y_sb[:, 512 * g + 128 * j : 512 * g + 128 * (j + 1)],
                                yps[:], gcol, None, bass.mybir.AluOpType.mult,
                            )
                        else:
                            nc.vector.tensor_scalar(
                                tmp[:, 128 * j : 128 * (j + 1)],
                                yps[:], gcol, None, bass.mybir.AluOpType.mult,
                            )
                    if e > 0:
                        nc.gpsimd.tensor_add(
                            y_sb[:, 512 * g : 512 * (g + 1)],
                            y_sb[:, 512 * g : 512 * (g + 1)],
                            tmp[:],
                        )

            y_sb = y_p.tile([128, B], f32, tag="y")

            # mm1(e0) first so ACT's gelu stream starts as early as possible
            h0 = emit_mm1(0)

            # ---- gate_T [E, B] = relu(gw^T @ xT + gb)  (relu on DVE) ----
            gw_sb = small_p.tile([128, E], dt_mm, tag="gw")
            nc.gpsimd.dma_start(gw_sb[:], gw_d[tl])
            gb_sb = small_p.tile([E, 1], f32, tag="gb")
            nc.gpsimd.dma_start(gb_sb[:], gb_d[tl])
            gate_T = gt_p.tile([E, B], dt_mm, tag="gateT")
            for c in range(NCH):
                gps = sps_p.tile([E, 512], f32, tag="sp")
                nc.tensor.matmul(
                    gps[:], gw_sb[:], xT[:, 512 * c : 512 * (c + 1)],
                    start=True, stop=True,
                )
                nc.vector.tensor_scalar(
                    gate_T[:, 512 * c : 512 * (c + 1)], gps[:],
                    gb_sb[:, 0:1], 0.0,
                    bass.mybir.AluOpType.add, bass.mybir.AluOpType.max,
                )
            # gate [B, E] per-block by PE-transposing gate_T back
            gate_sb = gt_p.tile([128, E * NBLK], f32, tag="gate")
            for blk in range(NBLK):
                tp = sps_p.tile([128, E], dt_mm, tag="sp", name=f"tp_{tl}_{blk}")
                nc.tensor.transpose(
                    tp[:], gate_T[:, 128 * blk : 128 * (blk + 1)], ident[:]
                )
                nc.vector.tensor_copy(gate_sb[:, E * blk : E * (blk + 1)], tp[:])

            emit_mm2(0, h0)
            for e in range(1, E):
                h_sb = emit_mm1(e)
                emit_mm2(e, h_sb)

            nc.sync.dma_start(
                y_d[:, tl, :].rearrange("(blk p) d -> p blk d", p=128),
                y_sb[:].rearrange("p (blk d) -> p blk d", blk=NBLK),
            )

    nc.compile()
    return nc


def get_program(dt_mm_name: str = "bfloat16"):
    key = ("nc", dt_mm_name)
    if key not in _CACHE:
        _install_ntff_hook()
        _CACHE[key] = _build(dt_mm_name)
    return _CACHE[key]


def _np_dt(dt_mm_name):
    if dt_mm_name == "bfloat16":
        import ml_dtypes

        return ml_dtypes.bfloat16
    return np.float32


def make_in_maps(x, W1, b1, W2, b2, gate_w_infer, gate_b_infer, dt_mm_name="bfloat16"):
    c = np.ascontiguousarray
    ndt = _np_dt(dt_mm_name)
    x = np.asarray(x, np.float32)
    W1 = np.asarray(W1, np.float32)
    b1 = np.asarray(b1, np.float32)
    W2 = np.asarray(W2, np.float32)
    gw = np.asarray(gate_w_infer, np.float32)
    gb = np.asarray(gate_b_infer, np.float32)
    ident = np.eye(E, dtype=np.float32)
    maps = []
    for i in range(N_CORES):
        s = slice(T_LOC * i, T_LOC * (i + 1))
        # xT[t, d, b] pre-transposed; b1 as [t, e, h%128, h//128]
        xTi = np.transpose(x[:, s, :], (1, 2, 0))
        b1i = np.transpose(b1[s].reshape(T_LOC, E, 4, 128), (0, 1, 3, 2))
        maps.append(
            {
                "xT": c(xTi.astype(ndt)),
                "w1": c(W1[s].astype(ndt)),
                "b1t": c(b1i),
                "w2": c(W2[s].astype(ndt)),
                "gw": c(gw[s].astype(ndt)),
                "gb": c(gb[s]),
                "ident": ident.astype(ndt),
            }
        )
    return maps


def kernel(x, W1, b1, W2, b2, gate_w_infer, gate_b_infer):
    from concourse.bass_utils import run_bass_kernel_spmd

    dt_mm_name = "bfloat16"
    nc = get_program(dt_mm_name)
    maps = make_in_maps(x, W1, b1, W2, b2, gate_w_infer, gate_b_infer, dt_mm_name)
    res = run_bass_kernel_spmd(nc, maps, list(range(N_CORES)))
    y = np.concatenate([res.results[i]["y"] for i in range(N_CORES)], axis=1)
    b2 = np.asarray(b2, np.float32)
    if np.any(b2):
        # b2 is all-zero for this problem's setup_inputs; handled host-side
        # for generality since the device kernel omits the b2 term.
        xf = np.asarray(x, np.float32)
        gate = np.einsum("btd,tde->bte", xf, np.asarray(gate_w_infer, np.float32))
        gate = np.maximum(gate + np.asarray(gate_b_infer, np.float32), 0.0)
        y = y + np.einsum("bte,ted->btd", gate, b2)
    return y, np.asarray(0.0, dtype=np.float32)


# revision 18
# speedup vs baseline: 1.2853x; 1.0255x over previous
"""Per-token sparse MoE kernel for Trainium2 (8 NeuronCores, Bass/Tile).

Problem: y[b,t,:] = sum_e relu(x[b,t]@gw[t])[e] * (gelu(x[b,t]@W1[t,e]+b1)@W2[t,e]+b2)
Shapes: x[2048,16,128], W1[16,4,128,512], W2[16,4,512,128], gates[16,128,4].

Sharding: the t dimension (16) is split across the 8 cores (2 t-values per
core). That makes the problem embarrassingly parallel (no collectives) and
each core only loads its own 1/8 of the weights (~2.1 MB in bf16) instead
of the full 33 MB, so the kernel is compute-bound rather than HBM-bound.

Host-side marshalling (inside kernel(), part of sharding): inputs are
sliced per-core, cast to the matmul dtype, and x is pre-transposed to
xT[t, d, b] so the device program needs no transpose/cast machinery for
its inputs.

Per-core device dataflow, per t:
  gate_T [E,B] = relu(gw^T @ xT)  (PE, gw stationary; ACT relu w/ bias)
  gate    [B,E]  by PE-transposing gate_T back (per 128-column block)
  h_T [H,B] = W1-slice^T @ xT     (PE, W1 stationary, 16 matmuls N=512)
  h = gelu(h_T + b1)              (ACT, exact-erf Gelu, per-partition bias)
  expert psum [Bblk,D] = h-block^T @ W2-block (PE, 4 accumulating matmuls)
  y += gate[:,e] * psum           (DVE tensor_scalar + batched adds)

b2 is all-zero in this problem; a host-side numpy correction covers the
general case.
"""

import contextlib
import ctypes
import sys
import types

import numpy as np

B, T, D, E, H = 2048, 16, 128, 4, 512
N_CORES = 8
T_LOC = T // N_CORES  # 2 t-values per core
NBLK = B // 128       # 16 b-blocks of 128
NCH = B // 512        # 4 b-chunks of 512 (matmul moving-operand max)

_CACHE: dict = {}


def _install_ntff_hook():
    """Provide antenv.axon_hooks (absent in this image) so that
    run_bass_kernel_spmd(trace=True) can capture NTFF profiles."""
    if "antenv.axon_hooks" in sys.modules:
        return
    try:
        lib = ctypes.CDLL("/opt/axon/libaxon_pjrt.so")
        if not hasattr(lib, "axon_start_nrt_profile"):
            hook = None
        else:
            lib.axon_start_nrt_profile.argtypes = [
                ctypes.POINTER(ctypes.c_int64),
                ctypes.c_size_t,
            ]
            lib.axon_start_nrt_profile.restype = ctypes.c_int64
            lib.axon_stop_nrt_profile.argtypes = [ctypes.c_char_p]
            lib.axon_stop_nrt_profile.restype = ctypes.c_int64

            @contextlib.contextmanager
            def hook(output_dir, device_ids):
                import jax

                jax.devices()
                if device_ids:
                    ids = (ctypes.c_int64 * len(device_ids))(*device_ids)
                    rc = lib.axon_start_nrt_profile(ids, len(device_ids))
                else:
                    rc = lib.axon_start_nrt_profile(None, 0)
                if rc != 0:
                    raise RuntimeError(f"axon_start_nrt_profile rc={rc}")
                try:
                    yield
                finally:
                    lib.axon_stop_nrt_profile(str(output_dir).encode())

        m = types.ModuleType("antenv.axon_hooks")
        m.get_axon_ntff_profile_hook = lambda: hook
        m.set_axon_ntff_profile_hook = lambda h: None
        sys.modules["antenv.axon_hooks"] = m
        import antenv

        antenv.axon_hooks = m
    except OSError:
        pass


def _build(dt_mm_name: str = "bfloat16"):
    """Build and compile the per-core Bass program. Same program on all cores.

    dt_mm_name selects the matmul-operand dtype (host pre-casts inputs):
      float32  — exact, but every matmul is a 2-pass HI/LO pair (slow)
      float32r — single-pass fp22-truncated reads (~2.6e-4 rel err)
      bfloat16 — single-pass + fast weight load (~4e-3 rel err)
    PSUM accumulation is fp32 in all cases.
    """
    import concourse.bass as bass
    import concourse.tile as tile
    from concourse import bacc, mybir

    dt_mm = getattr(mybir.dt, dt_mm_name)
    f32 = mybir.dt.float32
    AF = mybir.ActivationFunctionType

    nc = bacc.Bacc("TRN2", target_bir_lowering=False, debug=False, num_devices=N_CORES)

    xT_d = nc.dram_tensor("xT", [T_LOC, D, B], dt_mm, kind="ExternalInput").ap()
    w1_d = nc.dram_tensor("w1", [T_LOC, E, D, H], dt_mm, kind="ExternalInput").ap()
    b1_d = nc.dram_tensor("b1t", [T_LOC, E, 128, 4], f32, kind="ExternalInput").ap()
    w2_d = nc.dram_tensor("w2", [T_LOC, E, H, D], dt_mm, kind="ExternalInput").ap()
    gw_d = nc.dram_tensor("gw", [T_LOC, D, E], dt_mm, kind="ExternalInput").ap()
    gb_d = nc.dram_tensor("gb", [T_LOC, E], f32, kind="ExternalInput").ap()
    id_d = nc.dram_tensor("ident", [E, E], dt_mm, kind="ExternalInput").ap()
    y_d = nc.dram_tensor("y", [B, T_LOC, D], f32, kind="ExternalOutput").ap()

    with tile.TileContext(nc) as tc, contextlib.ExitStack() as ctx:
        ep = ctx.enter_context
        # SBUF pools
        const_p = ep(tc.tile_pool(name="const", bufs=1))
        xT_p = ep(tc.tile_pool(name="xT", bufs=2))
        h_p = ep(tc.tile_pool(name="h", bufs=2))
        w1_p = ep(tc.tile_pool(name="w1", bufs=3))
        w2_p = ep(tc.tile_pool(name="w2", bufs=3))
        y_p = ep(tc.tile_pool(name="y", bufs=2))
        tmp_p = ep(tc.tile_pool(name="tmp", bufs=4))
        gt_p = ep(tc.tile_pool(name="gt", bufs=2))
        small_p = ep(tc.tile_pool(name="small", bufs=4))
        # PSUM pools: hps 3x2 + sp 2 = 8 banks (gate/transpose psums share "sp")
        hps_p = ep(tc.tile_pool(name="hps", bufs=3, space="PSUM"))
        sps_p = ep(tc.tile_pool(name="sps", bufs=2, space="PSUM"))

        ident = const_p.tile([E, E], dt_mm)
        nc.gpsimd.dma_start(ident[:], id_d[:])

        # ---- software pipeline over (tl, e): mm1 runs one step ahead of
        # mm2 so the gelu (ACT) stream never starves, including across the
        # t boundary.
        xT_t, gate_t, y_t, w1e0_t, b1e0_t = {}, {}, {}, {}, {}

        def emit_t_head(tl):
            w1e0_t[tl] = w1_p.tile([128, H], dt_mm, tag="w1", name=f"w1_{tl}_0")
            nc.sync.dma_start(w1e0_t[tl][:], w1_d[tl, 0])
            b1e0_t[tl] = small_p.tile([128, 4], f32, tag="b1", name=f"b1_{tl}_0")
            nc.sync.dma_start(b1e0_t[tl][:], b1_d[tl, 0])
            xT = xT_p.tile([128, B], dt_mm, tag="xT", name=f"xT_{tl}")
            xT_t[tl] = xT
            for c in range(NCH):
                nc.sync.dma_start(
                    xT[:, 512 * c : 512 * (c + 1)], xT_d[tl, :, 512 * c : 512 * (c + 1)]
                )

        def emit_gate(tl):
            xT = xT_t[tl]
            gw_sb = small_p.tile([128, E], dt_mm, tag="gw", name=f"gw_{tl}")
            nc.gpsimd.dma_start(gw_sb[:], gw_d[tl])
            gb_sb = small_p.tile([E, 1], f32, tag="gb", name=f"gb_{tl}")
            nc.gpsimd.dma_start(gb_sb[:], gb_d[tl])
            gate_T = gt_p.tile([E, B], dt_mm, tag="gateT", name=f"gateT_{tl}")
            for c in range(NCH):
                gps = sps_p.tile([E, 512], f32, tag="sp", name=f"gps_{tl}_{c}")
                nc.tensor.matmul(
                    gps[:], gw_sb[:], xT[:, 512 * c : 512 * (c + 1)],
                    start=True, stop=True,
                )
                nc.vector.tensor_scalar(
                    gate_T[:, 512 * c : 512 * (c + 1)], gps[:],
                    gb_sb[:, 0:1], 0.0,
                    bass.mybir.AluOpType.add, bass.mybir.AluOpType.max,
                )
            # gate [B, E] per-block by PE-transposing gate_T back
            gate_sb = gt_p.tile([128, E * NBLK], f32, tag="gate", name=f"gate_{tl}")
            gate_t[tl] = gate_sb
            for blk in range(NBLK):
                tp = sps_p.tile([128, E], dt_mm, tag="sp", name=f"tp_{tl}_{blk}")
                nc.tensor.transpose(
                    tp[:], gate_T[:, 128 * blk : 128 * (blk + 1)], ident[:]
                )
                nc.vector.tensor_copy(gate_sb[:, E * blk : E * (blk + 1)], tp[:])

        def emit_mm1(tl, e):
            xT = xT_t[tl]
            if e == 0:
                w1_sb, b1_sb = w1e0_t[tl], b1e0_t[tl]
            else:
                w1_sb = w1_p.tile([128, H], dt_mm, tag="w1", name=f"w1_{tl}_{e}")
                nc.sync.dma_start(w1_sb[:], w1_d[tl, e])
                b1_sb = small_p.tile([128, 4], f32, tag="b1", name=f"b1_{tl}_{e}")
                nc.sync.dma_start(b1_sb[:], b1_d[tl, e])
            # h_T = gelu(W1slice^T @ xT + b1), laid out [128, (hb b)]
            h_sb = h_p.tile([128, 4 * B], dt_mm, tag="h", name=f"h_{tl}_{e}")
            for hb in range(4):
                for cc in range(2):  # two 1024-wide psum drains per hb
                    hps = hps_p.tile(
                        [128, 1024], f32, tag="hps", name=f"hps_{tl}_{e}_{hb}_{cc}"
                    )
                    for half in range(2):
                        c = 2 * cc + half
                        nc.tensor.matmul(
                            hps[:, 512 * half : 512 * (half + 1)],
                            w1_sb[:, 128 * hb : 128 * (hb + 1)],
                            xT[:, 512 * c : 512 * (c + 1)],
                            start=True, stop=True,
                        )
                    nc.scalar.activation(
                        h_sb[:, B * hb + 1024 * cc : B * hb + 1024 * (cc + 1)],
                        hps[:], AF.Gelu, bias=b1_sb[:, hb : hb + 1],
                    )
            return h_sb

        def emit_mm2(tl, e, h_sb):
            gate_sb, y_sb = gate_t[tl], y_t[tl]
            w2_sb = w2_p.tile([128, H], dt_mm, tag="w2", name=f"w2_{tl}_{e}")
            nc.sync.dma_start(
                w2_sb[:].rearrange("p (hk d) -> p hk d", hk=4),
                w2_d[tl, e].rearrange("(hk p) d -> p hk d", p=128),
            )
            # expert out per 128-block, gated accumulate into y
            for g in range(4):  # groups of 4 blocks -> batched adds
                if e > 0:
                    tmp = tmp_p.tile([128, 512], f32, tag="tmp", name=f"tmp_{tl}_{e}_{g}")
                else:
                    tmp = None
                for j in range(4):
                    blk = 4 * g + j
                    yps = sps_p.tile([128, 128], f32, tag="sp", name=f"yps_{tl}_{e}_{blk}")
                    for hk in range(4):
                        nc.tensor.matmul(
                            yps[:],
                            h_sb[:, B * hk + 128 * blk : B * hk + 128 * (blk + 1)],
                            w2_sb[:, 128 * hk : 128 * (hk + 1)],
                            start=(hk == 0), stop=(hk == 3),
                        )
                    gcol = gate_sb[:, E * blk + e : E * blk + e + 1]
                    if e == 0:
                        nc.vector.tensor_scalar(
                            y_sb[:, 512 * g + 128 * j : 512 * g + 128 * (j + 1)],
                            yps[:], gcol, None, bass.mybir.AluOpType.mult,
                        )
                    else:
                        nc.vector.tensor_scalar(
                            tmp[:, 128 * j : 128 * (j + 1)],
                            yps[:], gcol, None, bass.mybir.AluOpType.mult,
                        )
                if e > 0:
                    nc.gpsimd.tensor_add(
                        y_sb[:, 512 * g : 512 * (g + 1)],
                        y_sb[:, 512 * g : 512 * (g + 1)],
                        tmp[:],
                    )

        def emit_store(tl):
            nc.sync.dma_start(
                y_d[:, tl, :].rearrange("(blk p) d -> p blk d", p=128),
                y_t[tl][:].rearrange("p (blk d) -> p blk d", blk=NBLK),
            )

        emit_t_head(0)
        y_t[0] = y_p.tile([128, B], f32, tag="y", name="y_0")
        h_prev = emit_mm1(0, 0)
        emit_gate(0)
        prev = (0, 0, h_prev)
        steps = [(0, 1), (0, 2), (0, 3), (1, 0), (1, 1), (1, 2), (1, 3)]
        for tl, e in steps:
            if (tl, e) == (0, 3):
                emit_t_head(1)  # prefetch t1 inputs well before they're needed
            if (tl, e) == (1, 0):
                y_t[1] = y_p.tile([128, B], f32, tag="y", name="y_1")
            h_cur = emit_mm1(tl, e)
            if (tl, e) == (1, 0):
                emit_gate(1)
            emit_mm2(*prev)
            if (tl, e) == (1, 0):
                emit_store(0)
            prev = (tl, e, h_cur)
        emit_mm2(*prev)
        emit_store(1)

    nc.compile()
    return nc


def get_program(dt_mm_name: str = "bfloat16"):
    key = ("nc", dt_mm_name)
    if key not in _CACHE:
        _install_ntff_hook()
        _CACHE[key] = _build(dt_mm_name)
    return _CACHE[key]


def _np_dt(dt_mm_name):
    if dt_mm_name == "bfloat16":
        import ml_dtypes

        return ml_dtypes.bfloat16
    return np.float32


def make_in_maps(x, W1, b1, W2, b2, gate_w_infer, gate_b_infer, dt_mm_name="bfloat16"):
    c = np.ascontiguousarray
    ndt = _np_dt(dt_mm_name)
    x = np.asarray(x, np.float32)
    W1 = np.asarray(W1, np.float32)
    b1 = np.asarray(b1, np.float32)
    W2 = np.asarray(W2, np.float32)
    gw = np.asarray(gate_w_infer, np.float32)
    gb = np.asarray(gate_b_infer, np.float32)
    ident = np.eye(E, dtype=np.float32)
    maps = []
    for i in range(N_CORES):
        s = slice(T_LOC * i, T_LOC * (i + 1))
        # xT[t, d, b] pre-transposed; b1 as [t, e, h%128, h//128]
        xTi = np.transpose(x[:, s, :], (1, 2, 0))
        b1i = np.transpose(b1[s].reshape(T_LOC, E, 4, 128), (0, 1, 3, 2))
        maps.append(
            {
                "xT": c(xTi.astype(ndt)),
                "w1": c(W1[s].astype(ndt)),
                "b1t": c(b1i),
                "w2": c(W2[s].astype(ndt)),
                "gw": c(gw[s].astype(ndt)),
                "gb": c(gb[s]),
                "ident": ident.astype(ndt),
            }
        )
    return maps


def kernel(x, W1, b1, W2, b2, gate_w_infer, gate_b_infer):
    from concourse.bass_utils import run_bass_kernel_spmd

    dt_mm_name = "bfloat16"
    nc = get_program(dt_mm_name)
    maps = make_in_maps(x, W1, b1, W2, b2, gate_w_infer, gate_b_infer, dt_mm_name)
    res = run_bass_kernel_spmd(nc, maps, list(range(N_CORES)))
    y = np.concatenate([res.results[i]["y"] for i in range(N_CORES)], axis=1)
    b2 = np.asarray(b2, np.float32)
    if np.any(b2):
        # b2 is all-zero for this problem's setup_inputs; handled host-side
        # for generality since the device kernel omits the b2 term.
        xf = np.asarray(x, np.float32)
        gate = np.einsum("btd,tde->bte", xf, np.asarray(gate_w_infer, np.float32))
        gate = np.maximum(gate + np.asarray(gate_b_infer, np.float32), 0.0)
        y = y + np.einsum("bte,ted->btd", gate, b2)
    return y, np.asarray(0.0, dtype=np.float32)
